# revision 55
# baseline (speedup 1.0000x reference)
"""Trainium2 Bass kernel for BiDAF-style bidirectional attention.

Reference computation (per batch element n; M=1 folded away):
    s[i,j]  = h[i].w_h + u[j].w_u + (h[i]*u[j]).w_hu + b      [JX, JQ]
    a_u     = softmax_j(s);     u_a[i] = sum_j a_u[i,j] u[j]   (c2q)
    a_h     = softmax_i(max_j s);  h_a = sum_i a_h[i] h[i]     (q2c)
    out     = concat(h, u_a, h*u_a, h*h_a)                     [JX, 4D]

Sharding: data-parallel over batch N=8, one NeuronCore per batch element.
alpha_b drops out (both softmaxes are shift-invariant); accepted but unused.
The first output slice is h verbatim, so the device computes and stores only
[u_a | h*u_a | h*h_a] ([JX, 3D]); the host writes the h slice during unshard.

Per-core dataflow (i = context position, j = query position, d = feature):
  - h/u/alpha_w dram tensors are declared float32r (same bit layout as f32)
    so matmuls and transposes touching them run in the fast replicated mode
    without any on-chip recast; the score pipeline (hT, uwT, ET, u) runs in
    bf16 (1-cycle/row matmuls; ~4e-4 rel err vs the 2e-2 tolerance).
  - a short burst of dense fp32 dummy matmuls leads the PE stream: the HAM
    clock monitor tracks MAC activity (transposes are pass-through and do
    not count), and the burst drives the 1.2 -> 2.4 GHz ramp by ~12us;
    dummy "heartbeat" matmuls through the block phase keep it from
    down-throttling mid-kernel.  A dummy Exp forces the 1.3us
    ACT_TABLE_LOAD to the front of the scalar queue.
  - input DMAs issue on one queue in strict priority order (h0, h1, u,
    alpha) - the 16 hardware DMA rings drain roughly in issue order, so a
    late-issued critical tile would land after all earlier bulk.
  - scores are computed TRANSPOSED in 4 blocks of 256 i-columns:
    s0T[j,i] = sum_d uwT[d,j] hT[d,i] over 4 d-chunks (bf16), h.w_h folded
    in via a K=1 matmul and u.w_u as the per-partition bias of the ScalarE
    Exp evict: ET = exp(sT) in bf16.  uw = u*w_hu and uwu are built from
    K=1 broadcast matmuls of a flat alpha_w row + single DVE ops.
  - per block: PE re-transposes ET, one 3D DVE reduce pair gives row
    maxes/sums; u_a = (ET_tile^T @ u_bf16) scaled by 1/rowsum; o3 = h*u_a;
    one [128 x 4KB] store per tile on the sync queue.  The first two
    blocks' scores are both computed before block 0's softmax so their
    stores pipeline back-to-back; later blocks offload o2 to DVE and o3 to
    GpSimd to keep ScalarE on the exp path.
  - q2c runs as a deferred burst after the last block's reduces (8 matmuls
    into one PSUM group), then h_a is normalized and broadcast with K=1
    matmuls; o4 = h*h_a on GpSimd/DVE, stored as 2-tile pairs.
"""

import numpy as np

N_B, M_B, JX, JQ, D = 8, 1, 1024, 128, 512
P = 128
NT = JX // P    # 8 i-tiles
KC = D // P     # 4 d-chunks
TPB = 2         # tiles per score block
NB = NT // TPB  # 4 blocks
IB = TPB * P    # 256 i-columns per block

_CACHE = {}


def _build_program():
    from contextlib import ExitStack

    import concourse.bass as bass
    import concourse.tile as tile
    from concourse import bacc, mybir
    from concourse.masks import make_identity

    f32 = mybir.dt.float32
    f32r = mybir.dt.float32r
    bf16 = mybir.dt.bfloat16
    EXP = mybir.ActivationFunctionType.Exp
    AX = mybir.AxisListType.X
    ds = bass.ds

    nc = bacc.Bacc("TRN2", target_bir_lowering=False, debug=False, num_devices=8)
    h_d = nc.dram_tensor("h", [JX, D], f32r, kind="ExternalInput").ap()
    u_d = nc.dram_tensor("u", [JQ, D], f32r, kind="ExternalInput").ap()
    aw_d = nc.dram_tensor("alpha_w", [3 * D], f32r, kind="ExternalInput").ap()
    out_d = nc.dram_tensor("out", [JX, 3 * D], f32, kind="ExternalOutput").ap()

    with tile.TileContext(nc) as tc, ExitStack() as ctx:
        consts = ctx.enter_context(tc.tile_pool(name="consts", bufs=1))
        stage = ctx.enter_context(tc.tile_pool(name="stage", bufs=6))
        # PSUM budget (8 banks): tp=2, s0=2 (shared with warmup), ua=2,
        # acc=1, hap=1
        ps = ctx.enter_context(tc.tile_pool(name="ps", bufs=2, space="PSUM"))

        # ---- input DMAs: small tensors first (a tiny DMA issued late lands
        # after all earlier bulk on the shared engine rings), then h tiles.
        h_all = consts.tile([P, NT * D], f32r)  # tile t: h[t*128+p, d]
        def h_load(eng, t0, nt):
            src = h_d[ds(t0 * P, nt * P), :].rearrange("(t p) d -> p t d", p=P)
            dst = h_all[:, ds(t0 * D, nt * D)].rearrange("p (t d) -> p t d", d=D)
            eng.dma_start(dst, src)
        h_load(nc.sync, 0, 1)
        w_cm = consts.tile([12, P], f32r)  # alpha_w chunk-major (contiguous)
        nc.sync.dma_start(w_cm[:], aw_d.rearrange("(c p) -> c p", p=P))
        w_flat = consts.tile([1, 3 * D], f32r)  # alpha_w on one partition
        nc.sync.dma_start(w_flat[:], aw_d.rearrange("(o w) -> o w", o=1))
        u_sb = consts.tile([JQ, D], f32r)
        nc.sync.dma_start(u_sb[:], u_d[:])
        h_load(nc.sync, 1, 1)
        h_load(nc.sync, 2, 2)
        h_load(nc.sync, 4, 2)
        h_load(nc.sync, 6, 2)

        # ---- PE warmup: 512-wide dummy f32r matmuls from the first free
        # cycle until all loads have landed, so the HAM clock ramp
        # (1.2 -> 2.4 GHz after ~5.7us of continuous PE activity) completes
        # right as the real dependency chain gets going.
        warm_f = consts.tile([P, D], f32)
        nc.gpsimd.memset(warm_f[:], 0.25)
        warm = consts.tile([P, D], f32r)
        nc.vector.tensor_copy(warm[:], warm_f[:])
        wp = ps.tile([P, D], f32, tag="s0")
        heart = [None]
        def warm_mm(n):
            # dense fp32 4-pass matmuls: the HAM clock monitor tracks MAC
            # activity (transposes are pass-through and do not count), so
            # only heavy matmuls drive the 1.2 -> 2.4 GHz ramp
            for _ in range(n):
                nc.tensor.matmul(
                    wp[:], warm_f[:, ds(0, P)], warm_f[:], start=True, stop=True,
                )
        def beat(n=1):
            # PE heartbeat: dummy matmuls on a dedicated PSUM bank keep the
            # HAM activity monitor fed through cross-engine stalls so the
            # clock never down-throttles mid-kernel
            for _ in range(n):
                nc.tensor.matmul(
                    heart[0][:], warm[:, ds(0, P)], warm[:], start=True, stop=True,
                )
        # dummy Exp: forces the 1.3us ACT_TABLE_LOAD to the front of the
        # scalar queue instead of the middle of the score critical path
        warm_e = consts.tile([1, 1], f32)
        nc.scalar.activation(warm_e[:], warm_f[ds(0, 1), ds(0, 1)], EXP)

        # ---- constants ----
        ident = consts.tile([P, P], f32)
        make_identity(nc, ident[:])
        ident_r = consts.tile([P, P], f32r)
        nc.vector.tensor_copy(ident_r[:], ident[:])
        ident_b = consts.tile([P, P], bf16)
        nc.vector.tensor_copy(ident_b[:], ident[:])
        ones_col = consts.tile([P, 1], f32)
        nc.vector.memset(ones_col[:], 1.0)
        ones_fr = consts.tile([1, P], f32)
        nc.vector.memset(ones_fr[:], 1.0)
        ones_row_r = consts.tile([1, P], f32r)
        nc.vector.tensor_copy(ones_row_r[:], ones_fr[:])

        hT_all = consts.tile([P, KC * JX], bf16)  # chunk k: hT[k*128+p, i]
        hT3 = hT_all[:].rearrange("p (k x) -> p k x", k=KC)
        hwh_row = consts.tile([1, JX], f32r)      # h.w_h as a row over i
        ET = consts.tile([JQ, JX], bf16)          # exp(s0T + uwu[j] + hwh[i])
        m_exp_r = consts.tile([P, NT], f32r)      # per i-tile: max_j ET
        z_rec = consts.tile([P, NT], f32)         # per i-tile: 1/sum_j ET
        hap = ps.tile([1, D], f32, tag="hap", bufs=1)

        def transpose_tile(t, evict):
            tp = ps.tile([P, KC * P], f32r, tag="tp")
            for k in range(KC):
                nc.tensor.transpose(
                    tp[:, ds(k * P, P)], h_all[:, ds(t * D + k * P, P)],
                    ident_r[:],
                )
            evict(hT3[:, :, ds(t * P, P)], tp[:].rearrange("p (k x) -> p k x", k=KC))

        def prep_weights():
            # w_cols[p, c] = alpha_w[c*128+p] via one PE transpose of w_cm;
            # broadcast w_u / w_hu across partitions with K=1 matmuls so
            # uw = u * w_hu and uwu = sum_d u[j,d] w_u[d] are single DVE ops
            wtp = ps.tile([P, 12], f32r, tag="acc", bufs=1)
            nc.tensor.transpose(wtp[:], w_cm[:], ident_r[ds(0, 12), ds(0, 12)])
            w_cols_r = consts.tile([P, 12], f32r)
            nc.vector.tensor_copy(w_cols_r[:], wtp[:])
            w_colsb = consts.tile([P, 12], bf16)
            nc.vector.tensor_copy(w_colsb[:], wtp[:])
            u_b = consts.tile([JQ, D], bf16)
            nc.gpsimd.tensor_copy(u_b[:], u_sb[:])
            wb_u = ps.tile([JQ, D], f32, tag="hap", bufs=1)
            nc.tensor.matmul(
                wb_u[:], ones_row_r[:, ds(0, JQ)], w_flat[:, ds(D, D)],
                start=True, stop=True,
            )
            wb_hu = ps.tile([JQ, D], f32, tag="acc", bufs=1)
            nc.tensor.matmul(
                wb_hu[:], ones_row_r[:, ds(0, JQ)], w_flat[:, ds(2 * D, D)],
                start=True, stop=True,
            )
            wp2 = ps.tile([P, D], f32, tag="hap", bufs=1)
            heart[0] = wp2
            uwu = consts.tile([JQ, 1], f32)
            uw_scr = stage.tile([JQ, D], f32, tag="stg")
            nc.vector.scalar_tensor_tensor(
                uw_scr[:], u_sb[:], 1.0, wb_u[:],
                op0=mybir.AluOpType.mult, op1=mybir.AluOpType.mult,
                accum_out=uwu[:],
            )
            uw = consts.tile([JQ, D], f32r)
            nc.vector.tensor_mul(uw[:], u_sb[:], wb_hu[:])
            return w_colsb, u_b, uw, uwu

        def prep_uwT(uw):
            pt = ps.tile([P, KC * P], f32r, tag="tp")
            for k in range(KC):
                nc.tensor.transpose(
                    pt[:, ds(k * P, P)], uw[:, ds(k * P, P)], ident_r[:]
                )
            uwT = consts.tile([P, KC * JQ], bf16)
            nc.scalar.copy(uwT[:, ds(0, 2 * JQ)], pt[:, ds(0, 2 * P)])
            nc.vector.tensor_copy(uwT[:, ds(2 * JQ, 2 * JQ)], pt[:, ds(2 * P, 2 * P)])
            return uwT

        def block_scores(t0, nt, w_colsb, uwT):
            ib = nt * P
            blk = ds(t0 * P, ib)
            hp = ps.tile([1, ib], f32, tag="acc", bufs=1)
            for k in range(KC):
                nc.tensor.matmul(
                    hp[:], w_colsb[:, ds(k, 1)], hT_all[:, ds(k * JX + t0 * P, ib)],
                    start=(k == 0), stop=(k == KC - 1),
                )
            nc.scalar.copy(hwh_row[:, blk], hp[:])
            sp = ps.tile([JQ, ib], f32, tag="s0")
            for k in range(KC):
                nc.tensor.matmul(
                    sp[:], uwT[:, ds(k * JQ, JQ)], hT_all[:, ds(k * JX + t0 * P, ib)],
                    start=(k == 0), stop=False,
                )
            nc.tensor.matmul(
                sp[:], ones_row_r[:], hwh_row[:, blk], start=False, stop=True
            )
            return sp

        def block_softmax_c2q(t0, nt, sp, uwu, latency=False, tail_hook=None):
            blk = ds(t0 * P, nt * P)
            nc.scalar.activation(ET[:, blk], sp[:], EXP, bias=uwu[:])
            et = ps.tile([P, nt * P], bf16, tag="tp")
            for q in range(nt):
                t = t0 + q
                nc.tensor.transpose(
                    et[:, ds(q * P, P)], ET[:, ds(t * P, P)], ident_b[:]
                )
            beat(1)
            et3 = et[:].rearrange("p (q x) -> p q x", q=nt)
            nc.vector.reduce_max(m_exp_r[:, ds(t0, nt)], et3, axis=AX)
            zsum = stage.tile([P, nt], f32, tag="zs")
            nc.vector.reduce_sum(zsum[:], et3, axis=AX)
            nc.vector.reciprocal(z_rec[:, ds(t0, nt)], zsum[:])
            ups = []
            for q in range(nt):
                t = t0 + q
                up = ps.tile([P, D], f32, tag="ua")
                nc.tensor.matmul(
                    up[:], ET[:, ds(t * P, P)], u_b[:], start=True, stop=True
                )
                ups.append(up)
            beat(2)
            if tail_hook is not None:
                tail_hook()
            for q in range(nt):
                t = t0 + q
                up = ups[q]
                stg = stage.tile([P, 2 * D], f32, tag="stg")
                if latency:
                    nc.scalar.mul(stg[:, ds(0, D)], up[:], z_rec[:, ds(t, 1)])
                    nc.vector.scalar_tensor_tensor(
                        stg[:, ds(D, D)], up[:], z_rec[:, ds(t, 1)],
                        h_all[:, ds(t * D, D)],
                        op0=mybir.AluOpType.mult, op1=mybir.AluOpType.mult,
                    )
                else:
                    if t % 2 == 0:
                        nc.scalar.mul(stg[:, ds(0, D)], up[:], z_rec[:, ds(t, 1)])
                    else:
                        nc.vector.tensor_scalar_mul(
                            stg[:, ds(0, D)], up[:], z_rec[:, ds(t, 1)]
                        )
                    nc.gpsimd.tensor_mul(
                        stg[:, ds(D, D)], stg[:, ds(0, D)],
                        h_all[:, ds(t * D, D)],
                    )
                nc.sync.dma_start(out_d[ds(t * P, P), ds(0, 2 * D)], stg[:])

        # Software-pipelined emission: warmup leads; block sizes ramp
        # [1,1,2,2,2] so the first store issues as early as possible; the
        # next block's transposes slot between a block's score matmuls and
        # its softmax tail to hide Exp/reduce latency.
        dve = nc.vector.tensor_copy
        sca = nc.scalar.copy
        BL = [(0, 2), (2, 2), (4, 2), (6, 2)]
        evs = {0: dve, 1: dve, 2: dve, 3: dve, 4: sca, 5: dve, 6: sca, 7: dve}  # t4/t6 off DVE

        mrow = consts.tile([P, 1], f32)
        rzq = consts.tile([1, 1], f32)
        ha_sum = consts.tile([1, D], f32)
        ha_row = consts.tile([1, D], f32r)
        zqp = [None]
        def q2c_partial():
            # tiles 0-5: their q2c weights are final once block 2 reduces,
            # so most of the accumulation runs before the last block
            for t in range(6):
                nc.tensor.matmul(
                    hap[:], m_exp_r[:, ds(t, 1)], h_all[:, ds(t * D, D)],
                    start=(t == 0), stop=False, skip_group_check=True,
                )

        def q2c_tail():
            for t in range(6, NT):
                nc.tensor.matmul(
                    hap[:], m_exp_r[:, ds(t, 1)], h_all[:, ds(t * D, D)],
                    start=False, stop=(t == NT - 1), skip_group_check=True,
                )
            nc.vector.reduce_sum(mrow[:], m_exp_r[:], axis=AX)
            zq_ps = ps.tile([1, 1], f32, tag="acc", bufs=1)
            nc.tensor.matmul(zq_ps[:], mrow[:], ones_col[:], start=True, stop=True)
            nc.vector.reciprocal(rzq[:], zq_ps[:])
            nc.vector.tensor_copy(ha_sum[:], hap[:])
            nc.scalar.mul(ha_row[:], ha_sum[:], rzq[:])

        warm_mm(3)
        w_colsb, u_b, uw, uwu = prep_weights()
        transpose_tile(0, dve)
        transpose_tile(1, dve)
        warm_mm(1)
        uwT = prep_uwT(uw)
        transpose_tile(2, sca)
        warm_mm(1)
        transpose_tile(3, dve)
        sp0 = block_scores(0, 2, w_colsb, uwT)
        sp1 = block_scores(2, 2, w_colsb, uwT)
        block_softmax_c2q(0, 2, sp0, uwu, latency=True)
        transpose_tile(4, sca)
        transpose_tile(5, dve)
        warm_mm(1)
        sp2 = block_scores(4, 2, w_colsb, uwT)
        beat(1)
        block_softmax_c2q(2, 2, sp1, uwu)
        transpose_tile(6, sca)
        transpose_tile(7, dve)
        beat(1)
        sp3 = block_scores(6, 2, w_colsb, uwT)
        beat(1)
        block_softmax_c2q(4, 2, sp2, uwu)
        q2c_partial()
        block_softmax_c2q(6, 2, sp3, uwu, tail_hook=q2c_tail)

        # ---- o4 = h * h_a ----
        bc = ps.tile([P, D], f32, tag="acc", bufs=1)
        nc.tensor.matmul(bc[:], ones_row_r[:], ha_row[:], start=True, stop=True)
        bc_sb = consts.tile([P, D], f32)  # gpsimd cannot read PSUM
        nc.scalar.copy(bc_sb[:], bc[:])

        for pair in range(NT // 2):
            t0 = 2 * pair
            o4 = stage.tile([P, 2 * D], f32, tag="o4")
            nc.gpsimd.tensor_mul(o4[:, ds(0, D)], h_all[:, ds(t0 * D, D)], bc_sb[:])
            nc.vector.tensor_mul(o4[:, ds(D, D)], h_all[:, ds((t0 + 1) * D, D)], bc[:])
            eng = nc.scalar if pair % 2 == 0 else nc.sync
            eng.dma_start(
                out_d[ds(t0 * P, 2 * P), ds(2 * D, D)].rearrange(
                    "(t p) d -> p t d", p=P
                ),
                o4[:].rearrange("p (t d) -> p t d", d=D),
            )

    nc.compile()
    return nc


def _get_nc():
    if "nc" not in _CACHE:
        _CACHE["nc"] = _build_program()
    return _CACHE["nc"]


def _assemble(h, results):
    """Host-side unshard: column 0 of the output is h verbatim."""
    out = np.empty((N_B, JX, 4 * D), dtype=np.float32)
    out[:, :, :D] = h
    for n in range(N_B):
        out[n, :, D:] = results[n]["out"]
    return out.reshape(N_B, M_B, JX, 4 * D)


def _ensure_axon_hooks_stub():
    # concourse imports antenv.axon_hooks when tracing is requested via env;
    # provide a no-op stub if the image lacks it so runs degrade gracefully.
    import sys
    import types

    try:
        import antenv.axon_hooks  # noqa: F401
    except ImportError:
        mod = types.ModuleType("antenv.axon_hooks")
        _hook = [None]
        mod.set_axon_ntff_profile_hook = lambda hook: _hook.__setitem__(0, hook)
        mod.get_axon_ntff_profile_hook = lambda: _hook[0]
        sys.modules["antenv.axon_hooks"] = mod


def kernel(h, u, alpha_w, alpha_b=None, **_unused):
    _ensure_axon_hooks_stub()
    from concourse.bass_utils import run_bass_kernel_spmd

    h = np.ascontiguousarray(np.asarray(h, dtype=np.float32)).reshape(N_B, JX, D)
    u = np.ascontiguousarray(np.asarray(u, dtype=np.float32)).reshape(N_B, JQ, D)
    alpha_w = np.ascontiguousarray(np.asarray(alpha_w, dtype=np.float32)).reshape(3 * D)

    nc = _get_nc()
    in_maps = [
        {"h": h[n], "u": u[n], "alpha_w": alpha_w} for n in range(N_B)
    ]
    res = run_bass_kernel_spmd(nc, in_maps, core_ids=list(range(N_B)))
    return _assemble(h, res.results)


# revision 56
# speedup vs baseline: 1.0213x; 1.0213x over previous
"""Trainium2 Bass kernel for BiDAF-style bidirectional attention.

Reference computation (per batch element n; M=1 folded away):
    s[i,j]  = h[i].w_h + u[j].w_u + (h[i]*u[j]).w_hu + b      [JX, JQ]
    a_u     = softmax_j(s);     u_a[i] = sum_j a_u[i,j] u[j]   (c2q)
    a_h     = softmax_i(max_j s);  h_a = sum_i a_h[i] h[i]     (q2c)
    out     = concat(h, u_a, h*u_a, h*h_a)                     [JX, 4D]

Sharding: data-parallel over batch N=8, one NeuronCore per batch element.
alpha_b drops out (both softmaxes are shift-invariant); accepted but unused.
The first output slice is h verbatim, so the device computes and stores only
[u_a | h*u_a | h*h_a] ([JX, 3D]); the host writes the h slice during unshard.

Per-core dataflow (i = context position, j = query position, d = feature):
  - h/u/alpha_w dram tensors are declared float32r (same bit layout as f32)
    so matmuls and transposes touching them run in the fast replicated mode
    without any on-chip recast; the score pipeline (hT, uwT, ET, u) runs in
    bf16 (1-cycle/row matmuls; ~4e-4 rel err vs the 2e-2 tolerance).
  - a short burst of dense fp32 dummy matmuls leads the PE stream: the HAM
    clock monitor tracks MAC activity (transposes are pass-through and do
    not count), and the burst drives the 1.2 -> 2.4 GHz ramp by ~12us;
    dummy "heartbeat" matmuls through the block phase keep it from
    down-throttling mid-kernel.  A dummy Exp forces the 1.3us
    ACT_TABLE_LOAD to the front of the scalar queue.
  - input DMAs issue on one queue in strict priority order (h0, h1, u,
    alpha) - the 16 hardware DMA rings drain roughly in issue order, so a
    late-issued critical tile would land after all earlier bulk.
  - scores are computed TRANSPOSED in 4 blocks of 256 i-columns:
    s0T[j,i] = sum_d uwT[d,j] hT[d,i] over 4 d-chunks (bf16), h.w_h folded
    in via a K=1 matmul and u.w_u as the per-partition bias of the ScalarE
    Exp evict: ET = exp(sT) in bf16.  uw = u*w_hu and uwu are built from
    K=1 broadcast matmuls of a flat alpha_w row + single DVE ops.
  - per block: PE re-transposes ET, one 3D DVE reduce pair gives row
    maxes/sums; u_a = (ET_tile^T @ u_bf16) scaled by 1/rowsum; o3 = h*u_a;
    one [128 x 4KB] store per tile on the sync queue.  The first two
    blocks' scores are both computed before block 0's softmax so their
    stores pipeline back-to-back; later blocks offload o2 to DVE and o3 to
    GpSimd to keep ScalarE on the exp path.
  - q2c runs as a deferred burst after the last block's reduces (8 matmuls
    into one PSUM group), then h_a is normalized and broadcast with K=1
    matmuls; o4 = h*h_a on GpSimd/DVE, stored as 2-tile pairs.
"""

import numpy as np

N_B, M_B, JX, JQ, D = 8, 1, 1024, 128, 512
P = 128
NT = JX // P    # 8 i-tiles
KC = D // P     # 4 d-chunks
TPB = 2         # tiles per score block
NB = NT // TPB  # 4 blocks
IB = TPB * P    # 256 i-columns per block

_CACHE = {}


def _build_program():
    from contextlib import ExitStack

    import concourse.bass as bass
    import concourse.tile as tile
    from concourse import bacc, mybir
    from concourse.masks import make_identity

    f32 = mybir.dt.float32
    f32r = mybir.dt.float32r
    bf16 = mybir.dt.bfloat16
    EXP = mybir.ActivationFunctionType.Exp
    AX = mybir.AxisListType.X
    ds = bass.ds

    nc = bacc.Bacc("TRN2", target_bir_lowering=False, debug=False, num_devices=8)
    h_d = nc.dram_tensor("h", [JX, D], f32r, kind="ExternalInput").ap()
    u_d = nc.dram_tensor("u", [JQ, D], f32r, kind="ExternalInput").ap()
    aw_d = nc.dram_tensor("alpha_w", [3 * D], f32r, kind="ExternalInput").ap()
    out_d = nc.dram_tensor("out", [JX, 3 * D], f32, kind="ExternalOutput").ap()

    with tile.TileContext(nc) as tc, ExitStack() as ctx:
        consts = ctx.enter_context(tc.tile_pool(name="consts", bufs=1))
        stage = ctx.enter_context(tc.tile_pool(name="stage", bufs=6))
        # PSUM budget (8 banks): tp=2, s0=2 (shared with warmup), ua=2,
        # acc=1, hap=1
        ps = ctx.enter_context(tc.tile_pool(name="ps", bufs=2, space="PSUM"))

        # ---- input DMAs: small tensors first (a tiny DMA issued late lands
        # after all earlier bulk on the shared engine rings), then h tiles.
        h_all = consts.tile([P, NT * D], f32r)  # tile t: h[t*128+p, d]
        def h_load(eng, t0, nt):
            src = h_d[ds(t0 * P, nt * P), :].rearrange("(t p) d -> p t d", p=P)
            dst = h_all[:, ds(t0 * D, nt * D)].rearrange("p (t d) -> p t d", d=D)
            eng.dma_start(dst, src)
        h_load(nc.sync, 0, 1)
        w_cm = consts.tile([12, P], f32r)  # alpha_w chunk-major (contiguous)
        nc.sync.dma_start(w_cm[:], aw_d.rearrange("(c p) -> c p", p=P))
        w_flat = consts.tile([1, 3 * D], f32r)  # alpha_w on one partition
        nc.sync.dma_start(w_flat[:], aw_d.rearrange("(o w) -> o w", o=1))
        u_sb = consts.tile([JQ, D], f32r)
        nc.sync.dma_start(u_sb[:], u_d[:])
        h_load(nc.sync, 1, 1)
        h_load(nc.sync, 2, 2)
        h_load(nc.sync, 4, 2)
        h_load(nc.sync, 6, 2)

        # ---- PE warmup: 512-wide dummy f32r matmuls from the first free
        # cycle until all loads have landed, so the HAM clock ramp
        # (1.2 -> 2.4 GHz after ~5.7us of continuous PE activity) completes
        # right as the real dependency chain gets going.
        warm_f = consts.tile([P, D], f32)
        nc.gpsimd.memset(warm_f[:], 0.25)
        warm = consts.tile([P, D], f32r)
        nc.vector.tensor_copy(warm[:], warm_f[:])
        wp = ps.tile([P, D], f32, tag="s0")
        heart = [None]
        def warm_mm(n):
            # dense fp32 4-pass matmuls: the HAM clock monitor tracks MAC
            # activity (transposes are pass-through and do not count), so
            # only heavy matmuls drive the 1.2 -> 2.4 GHz ramp
            for _ in range(n):
                nc.tensor.matmul(
                    wp[:], warm_f[:, ds(0, P)], warm_f[:], start=True, stop=True,
                )
        def beat(n=1):
            # PE heartbeat: dummy matmuls on a dedicated PSUM bank keep the
            # HAM activity monitor fed through cross-engine stalls so the
            # clock never down-throttles mid-kernel
            for _ in range(n):
                nc.tensor.matmul(
                    heart[0][:], warm[:, ds(0, P)], warm[:], start=True, stop=True,
                )
        # dummy Exp: forces the 1.3us ACT_TABLE_LOAD to the front of the
        # scalar queue instead of the middle of the score critical path
        warm_e = consts.tile([1, 1], f32)
        nc.scalar.activation(warm_e[:], warm_f[ds(0, 1), ds(0, 1)], EXP)

        # ---- constants ----
        ident = consts.tile([P, P], f32)
        make_identity(nc, ident[:])
        ident_r = consts.tile([P, P], f32r)
        nc.vector.tensor_copy(ident_r[:], ident[:])
        ident_b = consts.tile([P, P], bf16)
        nc.vector.tensor_copy(ident_b[:], ident[:])
        ones_col = consts.tile([P, 1], f32)
        nc.vector.memset(ones_col[:], 1.0)
        ones_fr = consts.tile([1, P], f32)
        nc.vector.memset(ones_fr[:], 1.0)
        ones_row_r = consts.tile([1, P], f32r)
        nc.vector.tensor_copy(ones_row_r[:], ones_fr[:])

        hT_all = consts.tile([P, KC * JX], bf16)  # chunk k: hT[k*128+p, i]
        hT3 = hT_all[:].rearrange("p (k x) -> p k x", k=KC)
        hwh_row = consts.tile([1, JX], f32r)      # h.w_h as a row over i
        ET = consts.tile([JQ, JX], bf16)          # exp(s0T + uwu[j] + hwh[i])
        m_exp_r = consts.tile([P, NT], f32r)      # per i-tile: max_j ET
        z_rec = consts.tile([P, NT], f32)         # per i-tile: 1/sum_j ET
        hap = ps.tile([1, D], f32, tag="hap", bufs=1)

        def transpose_tile(t, evict):
            tp = ps.tile([P, KC * P], f32r, tag="tp")
            for k in range(KC):
                nc.tensor.transpose(
                    tp[:, ds(k * P, P)], h_all[:, ds(t * D + k * P, P)],
                    ident_r[:],
                )
            evict(hT3[:, :, ds(t * P, P)], tp[:].rearrange("p (k x) -> p k x", k=KC))

        def prep_weights():
            # w_cols[p, c] = alpha_w[c*128+p] via one PE transpose of w_cm;
            # broadcast w_u / w_hu across partitions with K=1 matmuls so
            # uw = u * w_hu and uwu = sum_d u[j,d] w_u[d] are single DVE ops
            wtp = ps.tile([P, 12], f32r, tag="acc", bufs=1)
            nc.tensor.transpose(wtp[:], w_cm[:], ident_r[ds(0, 12), ds(0, 12)])
            w_cols_r = consts.tile([P, 12], f32r)
            nc.vector.tensor_copy(w_cols_r[:], wtp[:])
            w_colsb = consts.tile([P, 12], bf16)
            nc.vector.tensor_copy(w_colsb[:], wtp[:])
            u_b = consts.tile([JQ, D], bf16)
            nc.gpsimd.tensor_copy(u_b[:], u_sb[:])
            wb_u = ps.tile([JQ, D], f32, tag="hap", bufs=1)
            nc.tensor.matmul(
                wb_u[:], ones_row_r[:, ds(0, JQ)], w_flat[:, ds(D, D)],
                start=True, stop=True,
            )
            wb_hu = ps.tile([JQ, D], f32, tag="acc", bufs=1)
            nc.tensor.matmul(
                wb_hu[:], ones_row_r[:, ds(0, JQ)], w_flat[:, ds(2 * D, D)],
                start=True, stop=True,
            )
            wp2 = ps.tile([P, D], f32, tag="hap", bufs=1)
            heart[0] = wp2
            uwu = consts.tile([JQ, 1], f32)
            uw_scr = stage.tile([JQ, D], f32, tag="stg")
            nc.vector.scalar_tensor_tensor(
                uw_scr[:], u_sb[:], 1.0, wb_u[:],
                op0=mybir.AluOpType.mult, op1=mybir.AluOpType.mult,
                accum_out=uwu[:],
            )
            uw = consts.tile([JQ, D], f32r)
            nc.vector.tensor_mul(uw[:], u_sb[:], wb_hu[:])
            return w_colsb, u_b, uw, uwu

        def prep_uwT(uw):
            pt = ps.tile([P, KC * P], f32r, tag="tp")
            for k in range(KC):
                nc.tensor.transpose(
                    pt[:, ds(k * P, P)], uw[:, ds(k * P, P)], ident_r[:]
                )
            uwT = consts.tile([P, KC * JQ], bf16)
            nc.scalar.copy(uwT[:, ds(0, 2 * JQ)], pt[:, ds(0, 2 * P)])
            nc.vector.tensor_copy(uwT[:, ds(2 * JQ, 2 * JQ)], pt[:, ds(2 * P, 2 * P)])
            return uwT

        def block_scores(t0, nt, w_colsb, uwT):
            ib = nt * P
            blk = ds(t0 * P, ib)
            hp = ps.tile([1, ib], f32, tag="acc", bufs=1)
            for k in range(KC):
                nc.tensor.matmul(
                    hp[:], w_colsb[:, ds(k, 1)], hT_all[:, ds(k * JX + t0 * P, ib)],
                    start=(k == 0), stop=(k == KC - 1),
                )
            nc.scalar.copy(hwh_row[:, blk], hp[:])
            sp = ps.tile([JQ, ib], f32, tag="s0")
            for k in range(KC):
                nc.tensor.matmul(
                    sp[:], uwT[:, ds(k * JQ, JQ)], hT_all[:, ds(k * JX + t0 * P, ib)],
                    start=(k == 0), stop=False,
                )
            nc.tensor.matmul(
                sp[:], ones_row_r[:], hwh_row[:, blk], start=False, stop=True
            )
            return sp

        def block_softmax_c2q(t0, nt, sp, uwu, latency=False, tail_hook=None):
            blk = ds(t0 * P, nt * P)
            nc.scalar.activation(ET[:, blk], sp[:], EXP, bias=uwu[:])
            et = ps.tile([P, nt * P], bf16, tag="tp")
            for q in range(nt):
                t = t0 + q
                nc.tensor.transpose(
                    et[:, ds(q * P, P)], ET[:, ds(t * P, P)], ident_b[:]
                )
            beat(1)
            et3 = et[:].rearrange("p (q x) -> p q x", q=nt)
            nc.vector.reduce_max(m_exp_r[:, ds(t0, nt)], et3, axis=AX)
            zsum = stage.tile([P, nt], f32, tag="zs")
            nc.vector.reduce_sum(zsum[:], et3, axis=AX)
            nc.vector.reciprocal(z_rec[:, ds(t0, nt)], zsum[:])
            ups = []
            for q in range(nt):
                t = t0 + q
                up = ps.tile([P, D], f32, tag="ua")
                nc.tensor.matmul(
                    up[:], ET[:, ds(t * P, P)], u_b[:], start=True, stop=True
                )
                ups.append(up)
            beat(2)
            if tail_hook is not None:
                tail_hook()
            for q in range(nt):
                t = t0 + q
                up = ups[q]
                stg = stage.tile([P, 2 * D], f32, tag="stg")
                if latency:
                    nc.scalar.mul(stg[:, ds(0, D)], up[:], z_rec[:, ds(t, 1)])
                    nc.vector.scalar_tensor_tensor(
                        stg[:, ds(D, D)], up[:], z_rec[:, ds(t, 1)],
                        h_all[:, ds(t * D, D)],
                        op0=mybir.AluOpType.mult, op1=mybir.AluOpType.mult,
                    )
                else:
                    if t % 2 == 0:
                        nc.scalar.mul(stg[:, ds(0, D)], up[:], z_rec[:, ds(t, 1)])
                    else:
                        nc.vector.tensor_scalar_mul(
                            stg[:, ds(0, D)], up[:], z_rec[:, ds(t, 1)]
                        )
                    nc.gpsimd.tensor_mul(
                        stg[:, ds(D, D)], stg[:, ds(0, D)],
                        h_all[:, ds(t * D, D)],
                    )
                nc.sync.dma_start(out_d[ds(t * P, P), ds(0, 2 * D)], stg[:])

        # Software-pipelined emission: warmup leads; block sizes ramp
        # [1,1,2,2,2] so the first store issues as early as possible; the
        # next block's transposes slot between a block's score matmuls and
        # its softmax tail to hide Exp/reduce latency.
        dve = nc.vector.tensor_copy
        sca = nc.scalar.copy
        BL = [(0, 2), (2, 2), (4, 2), (6, 2)]
        evs = {0: dve, 1: dve, 2: dve, 3: dve, 4: sca, 5: dve, 6: sca, 7: dve}  # t4/t6 off DVE

        mrow = consts.tile([P, 1], f32)
        rzq = consts.tile([1, 1], f32)
        ha_sum = consts.tile([1, D], f32)
        ha_row = consts.tile([1, D], f32r)
        zqp = [None]
        def q2c_tail():
            # emitted right after the final block's reduces: the q2c
            # accumulation runs as one deferred burst, off the block cadence
            for t in range(NT):
                nc.tensor.matmul(
                    hap[:], m_exp_r[:, ds(t, 1)], h_all[:, ds(t * D, D)],
                    start=(t == 0), stop=(t == NT - 1),
                )
            nc.vector.reduce_sum(mrow[:], m_exp_r[:], axis=AX)
            zq_ps = ps.tile([1, 1], f32, tag="acc", bufs=1)
            nc.tensor.matmul(zq_ps[:], mrow[:], ones_col[:], start=True, stop=True)
            nc.vector.reciprocal(rzq[:], zq_ps[:])
            nc.vector.tensor_copy(ha_sum[:], hap[:])
            nc.scalar.mul(ha_row[:], ha_sum[:], rzq[:])

        warm_mm(3)
        w_colsb, u_b, uw, uwu = prep_weights()
        transpose_tile(0, dve)
        transpose_tile(1, dve)
        warm_mm(1)
        uwT = prep_uwT(uw)
        transpose_tile(2, sca)
        warm_mm(1)
        transpose_tile(3, dve)
        sp0 = block_scores(0, 2, w_colsb, uwT)
        sp1 = block_scores(2, 2, w_colsb, uwT)
        block_softmax_c2q(0, 2, sp0, uwu, latency=True)
        transpose_tile(4, sca)
        transpose_tile(5, dve)
        warm_mm(1)
        sp2 = block_scores(4, 2, w_colsb, uwT)
        beat(1)
        block_softmax_c2q(2, 2, sp1, uwu)
        transpose_tile(6, sca)
        transpose_tile(7, dve)
        beat(1)
        sp3 = block_scores(6, 2, w_colsb, uwT)
        beat(1)
        block_softmax_c2q(4, 2, sp2, uwu)
        block_softmax_c2q(6, 2, sp3, uwu, tail_hook=q2c_tail)

        # ---- o4 = h * h_a ----
        bc = ps.tile([P, D], f32, tag="acc", bufs=1)
        nc.tensor.matmul(bc[:], ones_row_r[:], ha_row[:], start=True, stop=True)
        bc_sb = consts.tile([P, D], f32)  # gpsimd cannot read PSUM
        nc.scalar.copy(bc_sb[:], bc[:])

        for pair in range(NT // 2):
            t0 = 2 * pair
            o4 = stage.tile([P, 2 * D], f32, tag="o4")
            nc.gpsimd.tensor_mul(o4[:, ds(0, D)], h_all[:, ds(t0 * D, D)], bc_sb[:])
            nc.vector.tensor_mul(o4[:, ds(D, D)], h_all[:, ds((t0 + 1) * D, D)], bc[:])
            eng = nc.scalar if pair % 2 == 0 else nc.sync
            eng.dma_start(
                out_d[ds(t0 * P, 2 * P), ds(2 * D, D)].rearrange(
                    "(t p) d -> p t d", p=P
                ),
                o4[:].rearrange("p (t d) -> p t d", d=D),
            )

    nc.compile()
    return nc


def _get_nc():
    if "nc" not in _CACHE:
        _CACHE["nc"] = _build_program()
    return _CACHE["nc"]


def _assemble(h, results):
    """Host-side unshard: column 0 of the output is h verbatim."""
    out = np.empty((N_B, JX, 4 * D), dtype=np.float32)
    out[:, :, :D] = h
    for n in range(N_B):
        out[n, :, D:] = results[n]["out"]
    return out.reshape(N_B, M_B, JX, 4 * D)


def _ensure_axon_hooks_stub():
    # concourse imports antenv.axon_hooks when tracing is requested via env;
    # provide a no-op stub if the image lacks it so runs degrade gracefully.
    import sys
    import types

    try:
        import antenv.axon_hooks  # noqa: F401
    except ImportError:
        mod = types.ModuleType("antenv.axon_hooks")
        _hook = [None]
        mod.set_axon_ntff_profile_hook = lambda hook: _hook.__setitem__(0, hook)
        mod.get_axon_ntff_profile_hook = lambda: _hook[0]
        sys.modules["antenv.axon_hooks"] = mod


def kernel(h, u, alpha_w, alpha_b=None, **_unused):
    _ensure_axon_hooks_stub()
    from concourse.bass_utils import run_bass_kernel_spmd

    h = np.ascontiguousarray(np.asarray(h, dtype=np.float32)).reshape(N_B, JX, D)
    u = np.ascontiguousarray(np.asarray(u, dtype=np.float32)).reshape(N_B, JQ, D)
    alpha_w = np.ascontiguousarray(np.asarray(alpha_w, dtype=np.float32)).reshape(3 * D)

    nc = _get_nc()
    in_maps = [
        {"h": h[n], "u": u[n], "alpha_w": alpha_w} for n in range(N_B)
    ]
    res = run_bass_kernel_spmd(nc, in_maps, core_ids=list(range(N_B)))
    return _assemble(h, res.results)
